# revision 41
# baseline (speedup 1.0000x reference)
"""Multi-head causal+padded attention on 8 TRN2 NeuronCores.

Strategy: data-parallel over batch (8 batches -> 8 cores, no collectives).
Per core, everything is computed in a transposed layout so that no PE
transposes of the attention matrix are needed, and the Q/V projections are
folded away algebraically:

  scores  = (q Wq^T)(k Wk^T)^T = q (Wq^T Wk) k^T
            -> G[h] = (Wk_h^T Wq_h)^T-matmul over kT   [e, tk]  (per head)
            -> S^T(kc,:) = G[h][:,kc]^T-block @ qT     [tk-part, tq-free]
  A^T     = exp(s * S^T)                   (key-pad mask via zeroed kT rows,
                                            causal diag via idb x tri matmul
                                            injected into PSUM)
  rowsum  = mkw^T @ A^T  (+ 65504*caserow outer product for degenerate rows)
  P[h]    = sum_kc k_nat[kc]^T-block @ A^T      [e, tq]   (raw masked keys!)
  Pn[h]   = P[h] * recip(rowsum)
  out^T   = sum_h (Wu_h Wv_h)^T-block @ Pn[h] + w2^T @ brows + bu

so the only PSUM->SBUF evacuations are the 8 G[h] tiles and the epilogue
multiplies. PE operands are f16 (host pre-rounds; f32 PSUM accumulate);
the four full-width key-block pairs of each long unit run their rowsum/P
consumes as fp8-e4m3 DoubleRow matmuls (A^T there is written fp8 by the
Exp; the flat softmax keeps A in [~0.03, 10], well inside e4m3 range), the
rest stays f16. The big input DMAs are split across the sync+scalar queues
ordered by first use, throwaway matmuls into the fin banks warm the PE
clock-gate during the DMA window, the Exp table is preloaded, and the whole
kernel is one software-pipelined stream: the short (first-query-half) unit's
scores are spread through the long unit, G[h+1] is projected inside head h,
the output projection accumulates into a persistent PSUM bank pair as each
Pn half completes, and the final head's epilogue is chunked 256-wide to
overlap recip/mult/fin/bias/DMA. Degenerate softmax rows (all keys masked /
padded query) are fixed exactly via a rank-2 correction (w2/brows,
host-computed selectors). Set K_DR=0 to fall back to all-f16 consumes
(rel err 2.7e-4 instead of 3.1e-3, ~3% slower).
"""

import ml_dtypes
import numpy as np

import concourse.bacc as bacc
import concourse.mybir as mybir
import concourse.tile as tile
from concourse.bass_utils import run_bass_kernel_spmd

F32 = mybir.dt.float32
F16 = mybir.dt.float16
BF16 = mybir.dt.bfloat16
F8E4 = mybir.dt.float8e4
DR = mybir.MatmulPerfMode.DoubleRow

import os
USE_DR = os.environ.get("K_DR", "1") == "1"

B, TQ, TK, E, H = 8, 1024, 1024, 128, 8
HE = H * E
SCALE = float(E) ** -0.5
TRI_NEG = -60000.0
CASE_BIG = 65504.0


def _build():
    nc = bacc.Bacc("TRN2", target_bir_lowering=False, debug=False)
    dp = nc.declare_dram_parameter
    d_qT = dp("qT", [E, TQ], F16, isOutput=False)
    d_kT = dp("kT", [E, TK], F16, isOutput=False)
    d_kn = dp("kn", [TK, E], F16, isOutput=False)
    d_mT = dp("mT", [E, HE], F16, isOutput=False)
    d_nuT = dp("nuT", [HE, E], F16, isOutput=False)
    d_mkw = dp("mkw", [TK, 128], F16, isOutput=False)
    d_mkw8 = dp("mkw8", [TK, 128], F8E4, isOutput=False)
    d_kn8 = dp("kn8", [TK, E], F8E4, isOutput=False)
    d_tri = dp("trineg", [128, 128], F16, isOutput=False)
    d_idb = dp("identb", [128, 128], F16, isOutput=False)
    d_case = dp("caserow", [1, TQ], F16, isOutput=False)
    d_onesc = dp("onesc", [1, 128], F16, isOutput=False)
    d_brow = dp("brows", [2, TQ], F16, isOutput=False)
    d_w2 = dp("w2", [2, E], F16, isOutput=False)
    d_bu = dp("bu", [E, 1], F32, isOutput=False)
    d_out = dp("out", [E, TQ], F32, isOutput=True)

    Exp = mybir.ActivationFunctionType.Exp
    Ident = mybir.ActivationFunctionType.Identity
    mult = mybir.AluOpType.mult
    mm = nc.tensor.matmul

    with tile.TileContext(nc) as tc:
        with (
            tc.tile_pool(name="const", bufs=1) as cp,
            tc.tile_pool(name="persist", bufs=1) as pp,
        ):
            # ---- input DMAs: one per queue so they land in parallel ----
            kTs = cp.tile([E, TK], F16, tag="kTs", name="kTs")
            mall = cp.tile([E, HE], F16, tag="mall", name="mall")
            qTs = cp.tile([E, TQ], F16, tag="qTs", name="qTs")
            knall = cp.tile([128, HE], F16, tag="knall", name="knall")
            # split + ordered by first use: G0 needs kT halves + mall[:,0:128];
            # the first attention unit is (0, half=1) so it reads qT[512:]
            nc.sync.dma_start(out=kTs[:, 0:512], in_=d_kT[:, 0:512])
            nc.scalar.dma_start(out=mall[:, 0:256], in_=d_mT[:, 0:256])
            nc.sync.dma_start(out=kTs[:, 512:TK], in_=d_kT[:, 512:TK])
            nc.scalar.dma_start(out=qTs[:, 512:TQ], in_=d_qT[:, 512:TQ])
            knall8 = cp.tile([128, HE], F8E4, tag="knall8", name="knall8")
            nc.sync.dma_start(
                out=knall8[:].rearrange("p (c e) -> p c e", c=8),
                in_=d_kn8.rearrange("(c p) e -> p c e", p=128),
            )
            mkwall8 = cp.tile([128, HE], F8E4, tag="mkwall8", name="mkwall8")
            nc.scalar.dma_start(
                out=mkwall8[:].rearrange("p (c e) -> p c e", c=8),
                in_=d_mkw8.rearrange("(c p) e -> p c e", p=128),
            )
            nc.scalar.dma_start(out=qTs[:, 0:512], in_=d_qT[:, 0:512])
            nc.sync.dma_start(
                out=knall[:].rearrange("p (c e) -> p c e", c=8),
                in_=d_kn.rearrange("(c p) e -> p c e", p=128),
            )
            nc.scalar.dma_start(out=mall[:, 256:HE], in_=d_mT[:, 256:HE])
            kn = [knall[:, kc * 128 : (kc + 1) * 128] for kc in range(8)]

            # ---- constants (gpsimd queue; attention consts first) ----
            tri = cp.tile([128, 128], F16, tag="tri", name="tri")
            nc.gpsimd.dma_start(out=tri[:], in_=d_tri[:])
            idb = cp.tile([128, 128], F16, tag="idb", name="idb")
            nc.gpsimd.dma_start(out=idb[:], in_=d_idb[:])
            mkwall = cp.tile([128, HE], F16, tag="mkwall", name="mkwall")
            nc.gpsimd.dma_start(
                out=mkwall[:].rearrange("p (c e) -> p c e", c=8),
                in_=d_mkw.rearrange("(c p) e -> p c e", p=128),
            )
            mkw = [mkwall[:, kc * 128 : (kc + 1) * 128] for kc in range(8)]
            case = cp.tile([1, TQ], F16, tag="case", name="case")
            nc.gpsimd.dma_start(out=case[:], in_=d_case[:])
            onesc = cp.tile([1, 128], F16, tag="onesc", name="onesc")
            nc.gpsimd.dma_start(out=onesc[:], in_=d_onesc[:])
            nuall = cp.tile([128, HE], F16, tag="nuall", name="nuall")
            nc.gpsimd.dma_start(
                out=nuall[:].rearrange("p (c e) -> p c e", c=8),
                in_=d_nuT.rearrange("(c p) e -> p c e", p=128),
            )
            nu = [nuall[:, h * 128 : (h + 1) * 128] for h in range(H)]
            brow = cp.tile([2, TQ], F16, tag="brow", name="brow")
            nc.gpsimd.dma_start(out=brow[:], in_=d_brow[:])
            w2 = cp.tile([2, 128], F16, tag="w2", name="w2")
            nc.gpsimd.dma_start(out=w2[:], in_=d_w2[:])
            bu = cp.tile([E, 1], F32, tag="bu", name="bu")
            nc.gpsimd.dma_start(out=bu[:], in_=d_bu[:])

            # ---- exp table preload (hide ~2.7us ACT_TABLE_LOAD) ----
            dmy = cp.tile([128, 1], F32, tag="dmy", name="dmy")
            dmyo = cp.tile([128, 1], F32, tag="dmyo", name="dmyo")
            nc.vector.memset(dmy[:], 0.0)
            nc.scalar.activation(out=dmyo[:], in_=dmy[:], func=Exp,
                                 bias=0.0, scale=1.0)
            # zero tile for PE warm-up matmuls (also during DMA window)
            zs = cp.tile([128, 512], F16, tag="zs", name="zs")
            nc.vector.memset(zs[:], 0.0)

            # ---- persistent activations ----
            G = [pp.tile([128, TK], F16, tag=f"G{h}", name=f"G{h}")
                 for h in range(H)]
            Pn = [pp.tile([128, TQ], F16, tag=f"Pn{h}", name=f"Pn{h}")
                  for h in range(H)]

            with (
                tc.tile_pool(name="stps", bufs=2, space="PSUM") as sp,
                tc.tile_pool(name="accps", bufs=2, space="PSUM") as ap_,
                tc.tile_pool(name="finps", bufs=1, space="PSUM") as fp_,
                tc.tile_pool(name="atp", bufs=12) as atp,
                tc.tile_pool(name="at2p", bufs=8) as at2p,
                tc.tile_pool(name="ssp", bufs=4) as ssp,
            ):
                n_evac = 0

                def evac(dst, src):
                    # all on vector: the scalar queue must stay clear for
                    # the exp stream
                    nonlocal n_evac
                    nc.vector.tensor_copy(dst, src)
                    n_evac += 1

                def proj_g(h):
                    for i, (a, b) in enumerate(((0, 512), (512, TK))):
                        ps = sp.tile([128, 512], F32, tag="st",
                                     name=f"psg{h}_{i}")
                        mm(ps[:], mall[:, h * 128 : (h + 1) * 128],
                           kTs[:, a:b], start=True, stop=True)
                        evac(G[h][:, a:b], ps[:])

                fin = [fp_.tile([128, 512], F32, tag=f"fin{i}",
                                name=f"fin{i}") for i in range(2)]

                # PE warm-up: throwaway matmuls into the fin banks (later
                # wiped by the first start=True accumulation)
                for i in range(6):
                    mm(fin[i % 2][:], zs[:, 0:128], zs[:],
                       start=True, stop=True)

                fin_started = [False, False]

                class Unit:
                    """One (head, query-half) softmax unit."""

                    def __init__(self, h, half):
                        self.h, self.half = h, half
                        self.q0 = half * 512
                        self.klast = 7 if half == 1 else 3
                        self.sum_ps = ap_.tile([128, 512], F32, tag="sum_ps",
                                               name=f"sum{h}_{half}")
                        self.out_ps = ap_.tile([128, 512], F32, tag="out_ps",
                                               name=f"out{h}_{half}")
                        self.ats = {}

                    def step(self, kc):
                        h, q0 = self.h, self.q0
                        r0 = max(kc * 128 - q0, 0)
                        n = 512 - r0
                        diag = q0 <= kc * 128 < q0 + 512
                        st = sp.tile([128, 512], F32, tag="st",
                                     name=f"st{h}_{self.half}_{kc}")
                        mm(st[:, r0:512], G[h][:, kc * 128 : (kc + 1) * 128],
                           qTs[:, q0 + r0 : q0 + 512], start=True,
                           stop=not diag)
                        if diag:
                            mm(st[:, r0 : r0 + 128], idb[:], tri[:],
                               start=False, stop=True)
                        if USE_DR and self.half == 1 and kc < 4:
                            # fp8 pair tiles for the DoubleRow consumes
                            if kc % 2 == 0:
                                at2 = at2p.tile([128, 1024], F8E4, tag="at2",
                                                name=f"at2_{h}_{kc // 2}")
                                self.ats[("p", kc // 2)] = at2
                            else:
                                at2 = self.ats[("p", kc // 2)]
                            j = kc % 2
                            nc.scalar.activation(
                                out=at2[:, j * 512 : j * 512 + 512],
                                in_=st[:], func=Exp, bias=0.0, scale=SCALE,
                            )
                        else:
                            at = atp.tile([128, 512], F16, tag="at",
                                          name=f"at{h}_{self.half}_{kc}")
                            self.ats[kc] = at
                            nc.scalar.activation(
                                out=at[:, 0:n], in_=st[:, r0:512], func=Exp,
                                bias=0.0, scale=SCALE,
                            )

                    def consume(self, kc, start=False):
                        r0 = max(kc * 128 - self.q0, 0)
                        n = 512 - r0
                        at = self.ats.pop(kc)
                        mm(self.sum_ps[:, r0:512], mkw[kc][:], at[:, 0:n],
                           start=start, stop=False)
                        mm(self.out_ps[:, r0:512], kn[kc][:], at[:, 0:n],
                           start=start, stop=(kc == self.klast))

                    def consume_pair(self, kp):
                        if not USE_DR:
                            self.consume(2 * kp, start=(kp == 0))
                            self.consume(2 * kp + 1)
                            return
                        # fp8 DoubleRow: two full key blocks per matmul
                        a = kp * 256
                        at2 = self.ats.pop(("p", kp))
                        rhs = at2[:].rearrange("p (two n) -> p two n", two=2)
                        mm(self.sum_ps[:], mkwall8[:, a : a + 256].rearrange(
                            "p (two m) -> p two m", two=2),
                           rhs, start=(kp == 0), stop=False, perf_mode=DR)
                        mm(self.out_ps[:], knall8[:, a : a + 256].rearrange(
                            "p (two m) -> p two m", two=2),
                           rhs, start=(kp == 0), stop=False, perf_mode=DR)

                    def case(self):
                        q0 = self.q0
                        mm(self.sum_ps[:], onesc[:], case[:, q0 : q0 + 512],
                           start=False, stop=True)

                    def epilogue(self):
                        h, q0 = self.h, self.q0
                        rb = ssp.tile([128, 512], F32, tag="rb",
                                      name=f"rb{h}_{self.half}")
                        nc.vector.reciprocal_approx_fast(out=rb[:],
                                                         in_=self.sum_ps[:])
                        nc.vector.tensor_tensor(
                            out=Pn[h][:, q0 : q0 + 512], in0=self.out_ps[:],
                            in1=rb[:], op=mult,
                        )

                    def fin(self, stop=False):
                        h, half, q0 = self.h, self.half, self.q0
                        mm(fin[half][:], nu[h][:], Pn[h][:, q0 : q0 + 512],
                           start=not fin_started[half], stop=stop)
                        fin_started[half] = True

                def emit_w2(half):
                    q0 = half * 512
                    mm(fin[half][:], w2[:], brow[:, q0 : q0 + 512],
                       start=not fin_started[half], stop=False)
                    fin_started[half] = True

                proj_g(0)
                uL = Unit(0, 1)
                pL = pS = None
                for h in range(H):
                    uS = Unit(h, 0)
                    nL = Unit(h + 1, 1) if h < H - 1 else None

                    # long unit: kc 0-2 were pre-stepped in the previous
                    # short unit (except for h == 0)
                    for kc in range(8):
                        if kc >= 3 or h == 0:
                            uL.step(kc)
                        if kc == 3 and pS is not None:
                            pS.epilogue()
                        elif kc == 4:
                            uL.consume_pair(0)
                        elif kc == 5:
                            if h < H - 1:
                                proj_g(h + 1)
                            if h == 0:
                                emit_w2(1)
                                emit_w2(0)
                            uS.step(0)
                        elif kc == 6:
                            uL.consume_pair(1)
                            uS.step(1)
                        elif kc == 7:
                            uL.consume(4)
                            uS.step(2)
                    uL.consume(5)
                    uL.consume(6)
                    uS.step(3)
                    uL.consume(7)
                    uL.case()
                    if pS is not None:
                        pS.fin()
                        pL.fin()
                    if nL is not None:
                        uS.consume(0, start=True)
                        uL.epilogue()
                        nL.step(0)
                        uS.consume(1)
                        nL.step(1)
                        uS.consume(2)
                        nL.step(2)
                        uS.consume(3)
                        uS.case()
                    else:
                        # last head: chunked (7,1) finale interleaved with
                        # the short unit's consumes
                        outsb = pp.tile([E, TQ], F32, tag="outsb",
                                        name="outsb")
                        rbL = ssp.tile([128, 512], F32, tag="rb",
                                       name="rbL_tail")
                        uS.consume(0, start=True)
                        for i, last in ((0, False), (1, True)):
                            a = i * 256
                            nc.vector.reciprocal_approx_fast(
                                out=rbL[:, a : a + 256],
                                in_=uL.sum_ps[:, a : a + 256])
                            nc.vector.tensor_tensor(
                                out=Pn[h][:, 512 + a : 768 + a],
                                in0=uL.out_ps[:, a : a + 256],
                                in1=rbL[:, a : a + 256], op=mult,
                            )
                            uS.consume(1 + i)
                            mm(fin[1][:, a : a + 256], nu[h][:],
                               Pn[h][:, 512 + a : 768 + a],
                               start=False, stop=last)
                        uS.consume(3)
                        nc.scalar.activation(
                            out=outsb[:, 512:TQ], in_=fin[1][:],
                            func=Ident, bias=bu[:, 0:1], scale=1.0,
                        )
                        nc.sync.dma_start(out=d_out[:, 512:TQ],
                                          in_=outsb[:, 512:TQ])
                        uS.case()
                    pL, pS = uL, uS
                    uL = nL

                # final tail: flush the last short unit in 256-wide
                # chunks to overlap recip/mult/fin/bias-add/DMA
                h7, q7 = pS.h, pS.q0
                rbt = ssp.tile([128, 512], F32, tag="rb", name="rb_tail")
                for i, last in ((0, False), (1, True)):
                    a = i * 256
                    nc.vector.reciprocal_approx_fast(
                        out=rbt[:, a : a + 256], in_=pS.sum_ps[:, a : a + 256])
                    nc.vector.tensor_tensor(
                        out=Pn[h7][:, q7 + a : q7 + a + 256],
                        in0=pS.out_ps[:, a : a + 256],
                        in1=rbt[:, a : a + 256], op=mult,
                    )
                    mm(fin[0][:, a : a + 256], nu[h7][:],
                       Pn[h7][:, q7 + a : q7 + a + 256],
                       start=False, stop=last)
                    nc.scalar.activation(
                        out=outsb[:, a : a + 256], in_=fin[0][:, a : a + 256],
                        func=Ident, bias=bu[:, 0:1], scale=1.0,
                    )
                    nc.sync.dma_start(out=d_out[:, a : a + 256],
                                      in_=outsb[:, a : a + 256])

    nc.compile()
    return nc


_NC = None


def _get_nc():
    global _NC
    if _NC is None:
        _NC = _build()
    return _NC


def _host_prep(q, k, mask_q, mask_k, Wq, Wk, Wv, Wu, bu):
    f16 = np.float16
    # fold projections: scores = q (Wq^T Wk) k^T ; out = (Wu_h Wv_h) (k^T A)
    mT = np.concatenate(
        [Wk[h * E : (h + 1) * E].T @ Wq[h * E : (h + 1) * E]
         for h in range(H)], axis=1)  # [E, H*E], col block h = Wk_h^T Wq_h
    nuT = np.concatenate(
        [(Wu[:, h * E : (h + 1) * E] @ Wv[h * E : (h + 1) * E]).T
         for h in range(H)], axis=0)  # [H*E, E], row block h = (Wu_h Wv_h)^T
    shared = {
        "mT": np.ascontiguousarray(mT).astype(f16),
        "nuT": np.ascontiguousarray(nuT).astype(f16),
        "trineg": (TRI_NEG * np.tril(np.ones((128, 128), np.float32), -1)
                   ).astype(f16),
        "identb": np.eye(128).astype(f16),
        "onesc": np.full((1, 128), CASE_BIG, f16),
        "bu": np.ascontiguousarray(bu[:, None]).astype(np.float32),
    }
    WuWv = (Wu @ Wv).astype(np.float32)
    in_maps = []
    for b in range(B):
        mq = mask_q[b, :, 0].astype(np.float32)
        mk = mask_k[b, :, 0].astype(np.float32)
        c01 = (np.cumsum(mk) >= 1.0).astype(np.float32)
        caseA = mq * c01
        b1 = mq * (1.0 - c01)
        b2 = 1.0 - mq
        s1m = 1.0 - mk
        denom = max(float(s1m.sum()), 1.0)
        wvecs = np.stack([s1m / denom, np.full(TK, 1.0 / TK, np.float32)],
                         axis=1)
        w2 = (wvecs.T.astype(np.float32) @ k[b]) @ WuWv.T
        km = k[b] * mk[:, None]
        m = dict(shared)
        m["qT"] = np.ascontiguousarray(q[b].T).astype(f16)
        m["kT"] = np.ascontiguousarray(km.T).astype(f16)
        m["kn"] = np.ascontiguousarray(km).astype(f16)
        mkb = np.ascontiguousarray(np.broadcast_to(mk[:, None], (TK, 128)))
        m["mkw"] = mkb.astype(f16)
        m["mkw8"] = mkb.astype(ml_dtypes.float8_e4m3)
        m["kn8"] = np.ascontiguousarray(km).astype(ml_dtypes.float8_e4m3)
        m["caserow"] = (CASE_BIG * (1.0 - caseA))[None, :].astype(f16)
        m["brows"] = np.stack([b1, b2]).astype(f16)
        m["w2"] = np.ascontiguousarray(w2).astype(f16)
        in_maps.append(m)
    return in_maps


def kernel(q, k, mask_q, mask_k, Wq, Wk, Wv, Wu, bu):
    nc = _get_nc()
    in_maps = _host_prep(q, k, mask_q, mask_k, Wq, Wk, Wv, Wu, bu)
    res = run_bass_kernel_spmd(nc, in_maps, list(range(B)))
    out = np.stack([np.ascontiguousarray(res.results[b]["out"].T)
                    for b in range(B)])
    return out.astype(np.float32)
